# revision 11
# baseline (speedup 1.0000x reference)
"""ChebyKAN layer on 8 Trainium2 NeuronCores (data-parallel over batch).

Computation:  out[b,o] = sum_{i,d} T_d(tanh(x)[b,i]) * C[i,o,d]
  - batch 32768 sharded 8 ways (4096 rows/core), coefficients replicated.
  - Per core: x-shard pre-transposed on host to [i=512, b=4096].
  - Feature basis change (free on host): instead of T_1..T_8, the device
    computes {T1, T2, T2*T1, T4, T4*T1, T4*T2, T4*T2*T1, T8} -- squarings
    (2v^2 via ACT Square(sqrt2*v)) plus pure DVE multiplies; coefficients
    are transformed on the host (well-conditioned, ~4x amplification).
  - PE contracts over (i,d) with C chunks [i=128, o=128] as the STATIONARY
    operand and feature tiles [i=128, b=512] as the moving operand; each
    weight load serves two 512-column matmuls (a post-schedule pass drops
    the duplicate InstLdweights the tile framework emits per matmul).
    Output accumulates as outT[o,b] in PSUM; the host transposes back.
  - PSUM: two banks per o-chunk, rotated over all 8 banks so copy-out
    (ACT Identity with per-partition bias + 1/C_SCALE) overlaps matmuls.
"""

import os
from functools import lru_cache

import numpy as np
import ml_dtypes

import concourse.bass as bass
import concourse.mybir as mybir
import concourse.tile as tile
from concourse import bacc
from concourse.bass_utils import run_bass_kernel_spmd

N_CORES = 8
BATCH, IN_F, OUT_F, DEG = 32768, 512, 512, 8
B_LOC = BATCH // N_CORES  # 4096
P = 128
N_ICHUNK = IN_F // P  # 4
N_KCHUNK = DEG * N_ICHUNK  # 32 (d=0 handled as a bias add at copy-out)
N_OCHUNK = OUT_F // P  # 4
BGRP = 1024  # batch columns resident per pipeline stage
N_GRP = B_LOC // BGRP  # 4
MM_NP = np.float16
# coefficients scaled up on host so fp16 C stays normal; undone at copy-out
C_SCALE = 1024.0
# 1 = drop redundant weight loads (2nd matmul of each pair reuses the array)
LDW_DEDUP = int(os.environ.get("CHEBY_LDW_DEDUP", "1"))


def _dedup_ldweights(nc):
    """Remove InstLdweights whose weights AP equals the previous load's.

    The tile scheduler emits one InstLdweights per InstMatmult even when
    consecutive matmuls share the stationary operand; the PE then spends
    ~128 cycles reloading identical weights between 512-cycle matmuls.
    Duplicates carry no sync_info, so they can be dropped outright.
    """
    import json as _json
    import concourse.mybir as _mybir

    removed = 0
    for blk in nc.m.functions[0].blocks:
        last = None
        keep = []
        for inst in blk.instructions:
            if isinstance(inst, _mybir.InstLdweights):
                d = _json.loads(_mybir.instruction_to_pretty_json_string(inst))
                key = _json.dumps(d["ins"][0], sort_keys=True)
                si = d.get("sync_info")
                clean = not (si and (si.get("on_wait") or si.get("on_update")))
                if key == last and clean:
                    removed += 1
                    continue
                last = key
            keep.append(inst)
        blk.instructions[:] = keep
    return removed


def _build_kernel(reps=1):
    f32 = mybir.dt.float32
    f16 = mybir.dt.float16
    nc = bacc.Bacc(
        "TRN2",
        target_bir_lowering=False,
        debug=False,
        num_devices=N_CORES,
    )
    xT = nc.declare_dram_parameter("xT", [IN_F, B_LOC], f32, isOutput=False)
    cw = nc.declare_dram_parameter("Cw", [N_KCHUNK * P, OUT_F], f16, isOutput=False)
    biasT = nc.declare_dram_parameter("biasT", [P, N_OCHUNK], f32, isOutput=False)
    outT = nc.declare_dram_parameter("outT", [OUT_F, B_LOC], f32, isOutput=True)

    xT_ap = xT[:, :].rearrange("(c p) b -> p c b", p=P)  # [128, 4, B_LOC]
    cw_ap = cw[:, :].rearrange("(k p) o -> p k o", p=P)  # [128, 32, 512]

    import contextlib

    with tile.TileContext(nc) as tc:
        with (
            tc.tile_pool(name="const", bufs=1) as const_pool,
            tc.tile_pool(name="xin", bufs=2) as xin_pool,
            tc.tile_pool(name="cheb", bufs=2) as cheb_pool,
            tc.tile_pool(name="ot", bufs=4) as out_pool,
            tc.tile_pool(name="ps", bufs=4, space="PSUM") as psum_pool,
        ):
            c_tile = const_pool.tile([P, N_KCHUNK, OUT_F], f16)
            # split the C load so early k-chunks land before the first matmuls
            nsplit = 4
            per = (N_KCHUNK + nsplit - 1) // nsplit
            for s in range(nsplit):
                k0, k1 = s * per, min((s + 1) * per, N_KCHUNK)
                nc.gpsimd.dma_start(out=c_tile[:, k0:k1, :], in_=cw_ap[:, k0:k1, :])
            b_tile = const_pool.tile([P, N_OCHUNK], f32)
            nc.gpsimd.dma_start(out=b_tile[:, :], in_=biasT[:, :])

            rep_ctx = (
                tc.For_i(
                    0, reps, 1,
                    hint_engines=(
                        mybir.EngineType.PE,
                        mybir.EngineType.Activation,
                        mybir.EngineType.DVE,
                    ),
                )
                if reps > 1
                else contextlib.nullcontext()
            )
            with rep_ctx:
                _kernel_body(nc, tc, xT_ap, c_tile, b_tile, outT,
                             xin_pool, cheb_pool, out_pool, psum_pool)
    if LDW_DEDUP:
        _dedup_ldweights(nc)
    nc.compile()
    return nc


def _kernel_body(nc, tc, xT_ap, c_tile, b_tile, outT,
                 xin_pool, cheb_pool, out_pool, psum_pool):
    f32 = mybir.dt.float32
    f16 = mybir.dt.float16
    ACT_F = mybir.ActivationFunctionType

    SQRT2 = float(np.sqrt(2.0))

    def sub1(o):  # o -= 1
        nc.vector.tensor_scalar(
            out=o, in0=o, scalar1=1.0, scalar2=None,
            op0=mybir.AluOpType.subtract,
        )

    def cheby_stage(g):
        """DMA x columns for group g, then compute the 8 basis features.

        Feature j: 0:T1  1:T2  2:T2*T1  3:T4  4:T4*T1  5:T4*T2
                   6:T4*T2*T1  7:T8   (squarings via ACT Square(sqrt2*v))
        """
        b0 = g * BGRP
        x_in = xin_pool.tile([P, N_ICHUNK, BGRP], f32)
        nc.sync.dma_start(out=x_in[:, :, :], in_=xT_ap[:, :, b0 : b0 + BGRP])
        Tb = cheb_pool.tile([P, DEG, N_ICHUNK, BGRP], f16)
        t = [Tb[:, j, :, :] for j in range(DEG)]
        nc.scalar.activation(out=t[0], in_=x_in[:, :, :], func=ACT_F.Tanh)
        nc.scalar.activation(out=t[1], in_=t[0], func=ACT_F.Square, scale=SQRT2)
        sub1(t[1])                            # T2 = 2*T1^2 - 1
        nc.vector.tensor_mul(t[2], t[1], t[0])   # T2*T1
        nc.scalar.activation(out=t[3], in_=t[1], func=ACT_F.Square, scale=SQRT2)
        sub1(t[3])                            # T4 = 2*T2^2 - 1
        nc.vector.tensor_mul(t[4], t[3], t[0])   # T4*T1
        nc.vector.tensor_mul(t[5], t[3], t[1])   # T4*T2
        nc.vector.tensor_mul(t[6], t[3], t[2])   # T4*T2*T1
        nc.scalar.activation(out=t[7], in_=t[3], func=ACT_F.Square, scale=SQRT2)
        sub1(t[7])                            # T8 = 2*T4^2 - 1
        return Tb

    def matmul_stage(g, Tb):
        b0 = g * BGRP
        for oc in range(N_OCHUNK):
            o0 = oc * P
            ps = [
                psum_pool.tile([P, P * N_ICHUNK], f32, space="PSUM",
                               tag=f"ps{h}", name=f"ps{h}")
                for h in range(2)
            ]
            for k in range(N_KCHUNK):
                j, c = divmod(k, N_ICHUNK)
                w = c_tile[:, k, o0 : o0 + P]
                for h in range(2):
                    nc.tensor.matmul(
                        ps[h][:, :],
                        w,
                        Tb[:, j, c, h * 512 : (h + 1) * 512],
                        start=(k == 0),
                        stop=(k == N_KCHUNK - 1),
                    )
            for h in range(2):
                o_sb = out_pool.tile([P, P * N_ICHUNK], f32)
                # out = psum / C_SCALE + biasT[:, oc]  (per-partition bias)
                nc.scalar.activation(
                    out=o_sb[:, :],
                    in_=ps[h][:, :],
                    func=ACT_F.Identity,
                    scale=1.0 / C_SCALE,
                    bias=b_tile[:, oc : oc + 1],
                )
                col = b0 + h * 512
                nc.sync.dma_start(
                    out=outT[o0 : o0 + P, col : col + 512], in_=o_sb[:, :]
                )

    # Software pipeline: emit cheby(g+1) before matmuls(g) so the ACT/DVE
    # FIFO queues never head-of-line-block the next group's tanh/recurrence
    # behind the current group's PSUM copy-outs.
    Tbs = {0: cheby_stage(0)}
    for g in range(N_GRP):
        if g + 1 < N_GRP:
            Tbs[g + 1] = cheby_stage(g + 1)
        matmul_stage(g, Tbs.pop(g))


@lru_cache(maxsize=4)
def _get_nc(reps=1):
    return _build_kernel(reps)


class Runner:
    """Persistent jitted runner mirroring bass2jax.run_bass_via_pjrt, reusable
    across calls (single jit cache entry) so repeated executions can be timed
    back-to-back without recompilation or host round-trips per call."""

    def __init__(self, nc):
        import jax
        import jax.numpy as jnp
        from jax.sharding import Mesh, PartitionSpec
        from jax.experimental.shard_map import shard_map
        from concourse import bass2jax
        from concourse import mybir as _mybir

        bass2jax.install_neuronx_cc_hook()
        self.jax = jax
        self.nc = nc
        partition_name = (
            nc.partition_id_tensor.name if nc.partition_id_tensor else None
        )
        in_names, out_names, out_avals = [], [], []
        for alloc in nc.m.functions[0].allocations:
            if not isinstance(alloc, _mybir.MemoryLocationSet):
                continue
            name = alloc.memorylocations[0].name
            if alloc.kind == "ExternalInput":
                if name != partition_name:
                    in_names.append(name)
            elif alloc.kind == "ExternalOutput":
                out_names.append(name)
                out_avals.append(
                    jax.core.ShapedArray(
                        tuple(alloc.tensor_shape), _mybir.dt.np(alloc.dtype)
                    )
                )
        self.in_names = list(in_names)
        self.out_names = out_names
        self.out_avals = out_avals
        n_params = len(in_names)
        all_names = in_names + out_names
        if partition_name is not None:
            all_names = all_names + [partition_name]

        def _body(*args):
            operands = list(args)
            if partition_name is not None:
                operands.append(bass2jax.partition_id_tensor())
            return tuple(
                bass2jax._bass_exec_p.bind(
                    *operands,
                    out_avals=tuple(out_avals),
                    in_names=tuple(all_names),
                    out_names=tuple(out_names),
                    lowering_input_output_aliases=(),
                    sim_require_finite=True,
                    sim_require_nnan=True,
                    nc=nc,
                )
            )

        devices = jax.devices()[:N_CORES]
        self.mesh = Mesh(np.asarray(devices), ("core",))
        in_specs = (PartitionSpec("core"),) * (n_params + len(out_names))
        out_specs = (PartitionSpec("core"),) * len(out_names)
        self.fn = jax.jit(
            shard_map(
                _body,
                mesh=self.mesh,
                in_specs=in_specs,
                out_specs=out_specs,
                check_rep=False,
            ),
            keep_unused=True,
        )

    def put_inputs(self, in_maps):
        import jax
        from jax.sharding import NamedSharding, PartitionSpec

        concat = [
            np.concatenate([np.asarray(m[name]) for m in in_maps], axis=0)
            for name in self.in_names
        ]
        for aval in self.out_avals:
            concat.append(
                np.zeros((N_CORES * aval.shape[0], *aval.shape[1:]), aval.dtype)
            )
        sh = NamedSharding(self.mesh, PartitionSpec("core"))
        return [jax.device_put(a, sh) for a in concat]

    def __call__(self, dev_inputs):
        return self.fn(*dev_inputs)

    def run_np(self, in_maps):
        outs = self(self.put_inputs(in_maps))
        return [
            {
                name: np.asarray(outs[i]).reshape(N_CORES, *self.out_avals[i].shape)[c]
                for i, name in enumerate(self.out_names)
            }
            for c in range(N_CORES)
        ]


@lru_cache(maxsize=1)
def _basis_transform():
    """Map Chebyshev coefficients (d=1..8) onto the device feature basis.

    Features (as polynomials in t): f0=T1, f1=T2, f2=T2*T1, f3=T4,
    f4=T4*T1, f5=T4*T2, f6=T4*T2*T1, f7=T8.  Returns (M, const) with
    T_d = sum_p M[p, d-1] * f_p + const[d-1].
    """
    import numpy.polynomial.polynomial as Pp
    from numpy.polynomial.chebyshev import cheb2poly

    def cheb(d):
        out = np.zeros(9)
        c = cheb2poly(np.eye(9)[d])
        out[: len(c)] = c
        return out

    t1 = np.zeros(9); t1[1] = 1.0
    t2, t4, t8 = cheb(2), cheb(4), cheb(8)
    feats = [
        t1, t2, Pp.polymul(t2[:3], t1[:2]), t4,
        Pp.polymul(t4[:5], t1[:2]), Pp.polymul(t4[:5], t2[:3]),
        Pp.polymul(Pp.polymul(t4[:5], t2[:3]), t1[:2]), t8,
    ]
    A = np.zeros((9, 8))
    for p, f in enumerate(feats):
        A[: len(f), p] = f
    M = np.zeros((8, 8))
    const = np.zeros(8)
    for d in range(1, 9):
        tgt = cheb(d)
        sol, *_ = np.linalg.lstsq(A, tgt, rcond=None)
        resid = tgt - A @ sol
        assert np.abs(resid[1:]).max() < 1e-9, (d, resid)
        M[:, d - 1] = sol
        const[d - 1] = resid[0]
    return M, const


def _prep_inputs(x: np.ndarray, coefficients: np.ndarray):
    x = np.asarray(x, dtype=np.float32)
    coefficients = np.asarray(coefficients, dtype=np.float32)
    # Transform coefficients into the device feature basis; constants from
    # each T_d expansion and the d=0 term fold into a per-output bias row.
    M, const = _basis_transform()
    Cd = coefficients[:, :, 1:].astype(np.float64)  # (I, O, 8)
    Cp = np.einsum("pd,iod->iop", M, Cd)  # (I, O, 8) feature coeffs
    bias = (
        coefficients[:, :, 0].astype(np.float64).sum(axis=0)
        + np.einsum("d,iod->o", const, Cd)
    ).astype(np.float32)
    biasT = np.ascontiguousarray(bias.reshape(N_OCHUNK, P).T)  # [128, 4]
    # chunk k = p*4+c is feature p, i-chunk c, laid out [i within chunk, o]
    c_main = (
        np.transpose(Cp, (2, 0, 1))
        .reshape(DEG, N_ICHUNK, P, OUT_F)
        .reshape(N_KCHUNK * P, OUT_F)
        * C_SCALE
    )
    c_all = np.ascontiguousarray(c_main).astype(MM_NP)

    in_maps = []
    for core in range(N_CORES):
        shard = x[core * B_LOC : (core + 1) * B_LOC]  # (4096, 512)
        xt = np.ascontiguousarray(shard.T)  # (512, 4096)
        in_maps.append({"xT": xt, "Cw": c_all, "biasT": biasT})
    return in_maps


@lru_cache(maxsize=4)
def _get_runner(reps=1):
    return Runner(_get_nc(reps))


def run_sharded(x, coefficients):
    """Run the 8-core kernel; returns the full (32768, 512) float32 output."""
    in_maps = _prep_inputs(x, coefficients)
    runner = _get_runner()
    results = runner.run_np(in_maps)
    out = np.empty((BATCH, OUT_F), dtype=np.float32)
    for c in range(N_CORES):
        out[c * B_LOC : (c + 1) * B_LOC, :] = results[c]["outT"].T
    return out


def _time_runner(runner, dev_in, iters):
    import time

    outs = runner(dev_in)  # warm up
    outs[0].block_until_ready()
    times = []
    for _ in range(iters):
        t0 = time.perf_counter()
        outs = runner(dev_in)
        outs[0].block_until_ready()
        times.append((time.perf_counter() - t0) * 1e9)
    return times


def bench(x, coefficients, iters=12, rep_a=3, rep_b=83):
    """Estimate per-invocation HW time from the slope between two on-device
    repeat counts (fixed ~66-107ms axon RPC overhead cancels). Interleaved
    rounds + median to reject the bimodal RPC jitter. Returns
    (slope_ns, times_a, times_b)."""
    in_maps = _prep_inputs(x, coefficients)
    ra, rb = _get_runner(rep_a), _get_runner(rep_b)
    dev_a = ra.put_inputs(in_maps)
    dev_b = rb.put_inputs(in_maps)
    ta, tb = [], []
    for _ in range(3):
        ta += _time_runner(ra, dev_a, iters // 3 + 1)
        tb += _time_runner(rb, dev_b, iters // 3 + 1)
    med = lambda t: sorted(t)[len(t) // 2]
    slope = (med(tb) - med(ta)) / (rep_b - rep_a)
    return slope, ta, tb


def kernel(x, coefficients):
    return run_sharded(x, coefficients)
